# revision 4
# baseline (speedup 1.0000x reference)
"""Trainium2 Bass kernel for nn_DGT_6485400616966 (soft decision tree forward).

Math (forward pass only):
  pred_z = x @ W_pred.T + b_pred                      [B, 1023]
  The straight-through/one-hot structure collapses: the output depends only on
  the argmax leaf of the tree AND layer, which equals a 10-level tree descent
  following sign(pred_z) at visited nodes (left if z >= 0).
  out = softmax(W_or[:, leaf]) ; std = clip(action_stds[:, leaf], -20, 2)

Device algorithm per core (8192 samples, data-parallel over 8 cores):
  1. PE: z = x @ W_pred.T in three fp32r passes (xh@wh + xh@wl + xl@wh) where
     hi/lo are an exact e8m11 split of the fp32 operands (fp32r on HW is
     e8m11; one pass alone flips ~38 argmax rows, three passes flip none).
     x tiles are the stationary operand; W^T columns are the moving operand,
     in two 512-wide halves (N=512 keeps full fp32r rate), each half a
     separate PSUM tile so eviction starts after 6 matmuls.
  2. Eviction PSUM->SBUF per (btile, half): u = (z < 0) as fp16, contiguous
     writes. Split ~5:1 ACT saturated-sigmoid (Sigmoid(-1e30*z) is exactly
     {0,1}) / DVE tensor_scalar is_lt.
  3. DVE: bottom-up tree collapse r = r_e + u*(K + r_o - r_e), K = 2^(i-9),
     in fp16 on [128, NB, 2^i] chunk tensors. Node columns are stored
     BIT-REVERSED within each level so r_e / r_o are contiguous halves:
     every op is a packed-SBUF-fp16 scalar_tensor_tensor -> 4x DVE mode.
     The collapsed r is leaf/512; the x512 is folded into the int16 cast.
  4. GPSIMD ap_gather per chunk: table lookup T[class, leaf] with the 16
     classes replicated on partitions (tables bit-reversal-permuted on host;
     host pre-permutes rows by pi(p)=8*(p%16)+p//16 so indices are already
     wrapped and outputs land in natural order).
  5. PE transpose of gathered [128, 128] blocks (one block per 8 btiles,
     emitted two chunks late so the in-order PE queue never stalls) +
     contiguous DMA out.
  Schedule: a few junk warmup matmuls run during the initial DMA so the PE
  HAM clock gate is released before the real stream starts; weight DMAs are
  issued split per 512-column half in first-use order; chunk sizes are
  [2,6,8,8,8,8,8,8,6,2] so the pipeline starts early and drains fast.
"""

import sys

for _p in ("/opt/trn_rl_repo",):
    if _p not in sys.path:
        sys.path.insert(0, _p)

from contextlib import ExitStack

import numpy as np

import concourse.bacc as bacc
import concourse.bass as bass
import concourse.tile as tile
from concourse import mybir
from concourse.bass_utils import run_bass_kernel_spmd

HEIGHT = 10
IN_DIM = 256
OUT_DIM = 16
BATCH = 65536
N_CORES = 8
B_LOC = BATCH // N_CORES          # 8192 samples per core
NT = B_LOC // 128                 # 64 batch tiles of 128 samples
NODES = 1024                      # 1023 real + 1 pad
F32 = mybir.dt.float32
F32R = mybir.dt.float32r
BF16 = mybir.dt.bfloat16
FP16 = mybir.dt.float16
I16 = mybir.dt.int16

CHUNKS = [2, 6, 8, 8, 8, 8, 8, 8, 6, 2]   # btiles per chunk; sums to NT,
                                          # aligned to 8-btile output blocks
N_WARM = 16                               # junk matmuls to release HAM gate


def _build(nc, use_sign_path: bool):
    """Emit the per-core program. use_sign_path=True assumes b_pred == 0."""
    xTh = nc.dram_tensor("xTh", [IN_DIM, B_LOC], F32R, kind="ExternalInput")
    xTl = nc.dram_tensor("xTl", [IN_DIM, B_LOC], BF16, kind="ExternalInput")
    Wph = nc.dram_tensor("Wph", [IN_DIM, NODES], F32R, kind="ExternalInput")
    Wpl = nc.dram_tensor("Wpl", [IN_DIM, NODES], F32R, kind="ExternalInput")
    Wpb = nc.dram_tensor("Wpb", [IN_DIM, NODES], BF16, kind="ExternalInput")
    Tout = nc.dram_tensor("Tout", [128, NODES], F32, kind="ExternalInput")
    Tstd = nc.dram_tensor("Tstd", [128, NODES], F32, kind="ExternalInput")
    TH = nc.dram_tensor("TH", [128, NODES], F32, kind="ExternalInput")
    Ident = nc.dram_tensor("Ident", [128, 128], F32, kind="ExternalInput")
    out_o = nc.dram_tensor("out_o", [B_LOC, OUT_DIM], F32, kind="ExternalOutput")
    out_s = nc.dram_tensor("out_s", [B_LOC, OUT_DIM], F32, kind="ExternalOutput")

    with tile.TileContext(nc) as tc, ExitStack() as ctx:
        consts = ctx.enter_context(tc.tile_pool(name="consts", bufs=1))
        xpool = ctx.enter_context(tc.tile_pool(name="xpool", bufs=2))
        spool = ctx.enter_context(tc.tile_pool(name="spool", bufs=2))
        rpool = ctx.enter_context(tc.tile_pool(name="rpool", bufs=3))
        dpool = ctx.enter_context(tc.tile_pool(name="dpool", bufs=3))
        zpool = ctx.enter_context(
            tc.tile_pool(name="zpool", bufs=6, space=bass.MemorySpace.PSUM)
        )
        tpool = ctx.enter_context(
            tc.tile_pool(name="tpool", bufs=2, space=bass.MemorySpace.PSUM)
        )

        # weight tiles split per (ktile kk, 512-col half h) for fine-grained
        # DMA deps: the first matmul only waits on one 256 KB quarter.
        wh = [[consts.tile([128, 512], F32R, tag=f"wh{k}{h}", name=f"wh{k}{h}")
               for h in range(2)] for k in range(2)]
        wl = [[consts.tile([128, 512], F32R, tag=f"wl{k}{h}", name=f"wl{k}{h}")
               for h in range(2)] for k in range(2)]
        wb = [[consts.tile([128, 512], BF16, tag=f"wb{k}{h}", name=f"wb{k}{h}")
               for h in range(2)] for k in range(2)]
        t_out = consts.tile([128, NODES], F32)
        t_std = consts.tile([128, NODES], F32)
        ident = consts.tile([128, 128], F32)
        wfake = consts.tile([128, 512], BF16)
        th = None
        if not use_sign_path:
            th = consts.tile([128, NODES], F32)

        leaf_all = consts.tile([128, NT], FP16)
        leaf_i16 = consts.tile([128, NT], I16)
        r_out = consts.tile([128, NODES], F32)
        r_std = consts.tile([128, NODES], F32)

        o_view = out_o.rearrange("(t p f) c -> t p (f c)", t=8, p=128, f=8)
        s_view = out_s.rearrange("(t p f) c -> t p (f c)", t=8, p=128, f=8)

        A = mybir.AluOpType

        def dma_w(kk, h):
            ns = slice(512 * h, 512 * (h + 1))
            ks = slice(128 * kk, 128 * (kk + 1))
            nc.sync.dma_start(out=wh[kk][h], in_=Wph[ks, ns])
            nc.sync.dma_start(out=wl[kk][h], in_=Wpl[ks, ns])
            nc.sync.dma_start(out=wb[kk][h], in_=Wpb[ks, ns])

        # --- head: first weight quarter, then PE warmup on junk data ---
        nc.vector.memset(wfake, 0.0)
        nc.sync.dma_start(out=wh[0][0], in_=Wph[0:128, 0:512])
        zw = zpool.tile([128, 512], F32, tag="z")
        for i in range(N_WARM):
            nc.tensor.matmul(
                zw[:, 0:256], wfake[:, 0:128], wfake[:, 0:256],
                start=(i == 0), stop=(i == N_WARM - 1),
            )

        starts = np.cumsum([0] + CHUNKS).tolist()
        n_blocks = NT // 8
        blk_last_chunk = {}
        for b in range(n_blocks):
            for c, (t0, nb) in enumerate(zip(starts, CHUNKS)):
                if t0 + nb - 1 >= 8 * b and t0 + nb - 1 <= 8 * (b + 1) - 1:
                    blk_last_chunk[b] = c
        blocks_emitted = [False] * n_blocks

        def emit_block(b):
            # transpose block b's gathered [128, 128] table columns, DMA out
            blocks_emitted[b] = True
            rs_ = slice(128 * b, 128 * (b + 1))
            for rbuf, dview in ((r_out, o_view), (r_std, s_view)):
                pt = tpool.tile([128, 128], F32, tag="t", name="pt")
                nc.tensor.transpose(pt, rbuf[:, rs_], ident)
                rt = xpool.tile([128, 128], F32, tag="rt", name="rt", bufs=2)
                nc.vector.tensor_copy(out=rt, in_=pt)
                nc.sync.dma_start(out=dview[b], in_=rt)

        ev = 0  # eviction round-robin counter
        for c, (t0, NB) in enumerate(zip(starts, CHUNKS)):
            s_chunk = spool.tile([128, 8, NODES], FP16, tag="s")
            # stage x for this chunk: [128, 128*NB] per ktile/half
            hs = slice(128 * t0, 128 * (t0 + NB))
            xh = [xpool.tile([128, 128 * 8], F32R, tag=f"xh{kk}",
                             name=f"xh{kk}", bufs=2) for kk in range(2)]
            xl = [xpool.tile([128, 128 * 8], BF16, tag=f"xl{kk}",
                             name=f"xl{kk}", bufs=2) for kk in range(2)]
            for kk in range(2):
                ks = slice(128 * kk, 128 * (kk + 1))
                nc.sync.dma_start(out=xh[kk][:, : 128 * NB], in_=xTh[ks, hs])
                nc.sync.dma_start(out=xl[kk][:, : 128 * NB], in_=xTl[ks, hs])
            if c == 0:
                # remaining weights in first-use order, tables last
                nc.sync.dma_start(out=wl[0][0], in_=Wpl[0:128, 0:512])
                nc.sync.dma_start(out=wb[0][0], in_=Wpb[0:128, 0:512])
                dma_w(1, 0)
                nc.sync.dma_start(out=wh[0][1], in_=Wph[0:128, 512:1024])
                nc.sync.dma_start(out=wl[0][1], in_=Wpl[0:128, 512:1024])
                nc.sync.dma_start(out=wb[0][1], in_=Wpb[0:128, 512:1024])
                dma_w(1, 1)
                nc.sync.dma_start(out=t_out, in_=Tout[:, :])
                nc.sync.dma_start(out=t_std, in_=Tstd[:, :])
                nc.sync.dma_start(out=ident, in_=Ident[:, :])
                if th is not None:
                    nc.sync.dma_start(out=th, in_=TH[:, :])

            for k in range(NB):
                kb = slice(128 * k, 128 * (k + 1))
                for h in range(2):
                    ns = slice(512 * h, 512 * (h + 1))
                    z = zpool.tile([128, 512], F32, tag="z")
                    pair = 0
                    for kk in range(2):
                        for lhs, rhs in (
                            (xh[kk], wh[kk][h]),
                            (xh[kk], wl[kk][h]),
                            (xl[kk], wb[kk][h]),
                        ):
                            nc.tensor.matmul(
                                z, lhs[:, kb], rhs,
                                start=(pair == 0), stop=(pair == 5),
                            )
                            pair += 1
                    # u = (z < -b_pred); contiguous fp16 write of one half
                    if use_sign_path:
                        if ev % 6 != 5:
                            nc.scalar.activation(
                                out=s_chunk[:, k, ns], in_=z,
                                func=mybir.ActivationFunctionType.Sigmoid,
                                scale=-1e30,
                            )
                        else:
                            nc.vector.tensor_scalar(
                                out=s_chunk[:, k, ns], in0=z,
                                scalar1=0.0, scalar2=None, op0=A.is_lt,
                            )
                        ev += 1
                    else:
                        nc.vector.tensor_tensor(
                            out=s_chunk[:, k, ns], in0=z, in1=th[:, ns],
                            op=A.is_lt,
                        )

            # ---- bottom-up collapse; bit-reversed node layout makes r_e /
            # r_o contiguous halves, all ops packed-fp16 stt (4x DVE) ----
            r_prev = s_chunk[:, :NB, 511:1023]
            for i in range(8, -1, -1):
                n = 1 << i
                kconst = float(2.0 ** (i - 9))
                u_i = s_chunk[:, :NB, n - 1 : 2 * n - 1]
                r_e = r_prev[:, :, :n]
                r_o = r_prev[:, :, n : 2 * n]
                d_t = dpool.tile([128, NB, n], FP16, tag="d")
                nc.vector.scalar_tensor_tensor(
                    out=d_t, in0=r_o, scalar=kconst, in1=r_e,
                    op0=A.add, op1=A.subtract,
                )
                nc.vector.scalar_tensor_tensor(
                    out=d_t, in0=d_t, scalar=1.0, in1=u_i,
                    op0=A.mult, op1=A.mult,
                )
                if i > 0:
                    r_t = rpool.tile([128, NB, n], FP16, tag="r")
                    nc.vector.scalar_tensor_tensor(
                        out=r_t, in0=d_t, scalar=0.0, in1=r_e,
                        op0=A.add, op1=A.add,
                    )
                    r_prev = r_t
                else:
                    nc.vector.scalar_tensor_tensor(
                        out=leaf_all[:, t0 : t0 + NB],
                        in0=d_t[:, :, 0], scalar=0.0, in1=r_e[:, :, 0],
                        op0=A.add, op1=A.add,
                    )

            # leaf/512 -> int16 leaf index (x512 folded into the cast)
            cslice = slice(t0, t0 + NB)
            nc.vector.tensor_scalar(
                out=leaf_i16[:, cslice], in0=leaf_all[:, cslice],
                scalar1=512.0, scalar2=None, op0=A.mult,
            )
            # table gathers: R[16g+cls, j] = T[cls, leaf(sample 8j+g)]
            rs = slice(16 * t0, 16 * (t0 + NB))
            for tbl, rbuf in ((t_out, r_out), (t_std, r_std)):
                nc.gpsimd.ap_gather(
                    out_ap=rbuf[:, rs], in_ap=tbl,
                    idxs_ap=leaf_i16[:, cslice],
                    channels=128, num_elems=NODES, d=1, num_idxs=16 * NB,
                )
            for b in range(n_blocks):
                if not blocks_emitted[b] and blk_last_chunk[b] + 2 <= c:
                    emit_block(b)

        for b in range(n_blocks):
            if not blocks_emitted[b]:
                emit_block(b)

    nc.compile()
    return nc


_CACHE = {}


def _get_nc(use_sign_path: bool):
    key = use_sign_path
    if key not in _CACHE:
        nc = bacc.Bacc("TRN2", target_bir_lowering=False, debug=False)
        _CACHE[key] = _build(nc, use_sign_path)
    return _CACHE[key]


# Within each 128-row block, device partition p holds sample row PERM[p].
# PERM aligns the collapse output with ap_gather's wrapped index layout and
# makes the final outputs land in natural row order.
PERM = np.array([8 * (p % 16) + p // 16 for p in range(128)], dtype=np.int64)


def _bitrev(n_bits):
    """Bit-reversal permutation of range(2**n_bits)."""
    n = 1 << n_bits
    out = np.zeros(n, dtype=np.int64)
    for j in range(n):
        r = 0
        for b in range(n_bits):
            r |= ((j >> b) & 1) << (n_bits - 1 - b)
        out[j] = r
    return out


def _node_colperm():
    """colperm[stored] = natural padded-column; stored order is bit-reversed
    within each level so the device collapse reads contiguous halves."""
    perm = np.arange(NODES, dtype=np.int64)
    for lev in range(HEIGHT):
        off = (1 << lev) - 1
        perm[off : off + (1 << lev)] = off + _bitrev(lev)
    return perm


COLPERM = _node_colperm()
LEAFPERM = _bitrev(HEIGHT)  # natural leaf = LEAFPERM[stored leaf]


def _e8m11(x):
    """Round fp32 to the HW fp32r format (8-bit exp, 11-bit mantissa, RNE)."""
    u = np.ascontiguousarray(x, np.float32).view(np.uint32)
    low = u & np.uint32(0xFFF)
    base = u & np.uint32(0xFFFFF000)
    add = (low > 0x800) | ((low == 0x800) & ((u >> 12) & 1).astype(bool))
    return (base + np.where(add, np.uint32(0x1000), np.uint32(0))).view(np.float32)


def _split_hi_lo(a, lo_bf16=False):
    hi = _e8m11(a)
    lo = (a - hi).astype(np.float32)  # exactly e8m11-representable
    if lo_bf16:
        import ml_dtypes
        lo = lo.astype(ml_dtypes.bfloat16)
    return hi, lo


def _shard_xT(x_shard):
    """[8192, 256] sample rows -> permuted, transposed [256, 8192] device input."""
    xp = x_shard.reshape(NT, 128, IN_DIM)[:, PERM, :].reshape(B_LOC, IN_DIM)
    return np.ascontiguousarray(xp.T)


def _prepare(x, W_pred, b_pred, W_or, action_stds):
    x = np.ascontiguousarray(x, dtype=np.float32)
    W_pred = np.asarray(W_pred, dtype=np.float32)
    b_pred = np.asarray(b_pred, dtype=np.float32)
    W_or = np.asarray(W_or, dtype=np.float32)
    action_stds = np.asarray(action_stds, dtype=np.float32)

    n_int = 2**HEIGHT - 1
    Wp = np.zeros((IN_DIM, NODES), np.float32)
    Wp[:, :n_int] = W_pred.T
    Wp = Wp[:, COLPERM]  # bit-reversed node storage order per level
    Wph, Wpl = _split_hi_lo(Wp)
    import ml_dtypes
    Wpb = Wph.astype(ml_dtypes.bfloat16)
    # softmax over classes per leaf column, tables in stored-leaf order
    m = W_or.max(axis=0, keepdims=True)
    e = np.exp(W_or - m)
    t_out16 = (e / e.sum(axis=0, keepdims=True)).astype(np.float32)[:, LEAFPERM]
    t_std16 = np.clip(action_stds, -20.0, 2.0).astype(np.float32)[:, LEAFPERM]
    t_out = np.tile(t_out16, (8, 1))  # [128, 1024]
    t_std = np.tile(t_std16, (8, 1))
    th16 = np.zeros((NODES,), np.float32)
    th16[:n_int] = -b_pred
    th16 = th16[COLPERM]
    th = np.tile(th16[None, :], (128, 1))
    return x, Wph, Wpl, Wpb, t_out, t_std, th, bool(np.any(b_pred != 0.0))


def kernel(x, W_pred, b_pred, W_or, action_stds, _want_trace=False):
    x, Wph, Wpl, Wpb, t_out, t_std, th, b_nonzero = _prepare(
        x, W_pred, b_pred, W_or, action_stds
    )
    nc = _get_nc(use_sign_path=not b_nonzero)

    in_maps = []
    for c in range(N_CORES):
        shard = x[c * B_LOC : (c + 1) * B_LOC]
        xt = _shard_xT(shard)
        xth, xtl = _split_hi_lo(xt, lo_bf16=True)
        in_maps.append(
            {
                "xTh": xth,
                "xTl": xtl,
                "Wph": Wph,
                "Wpl": Wpl,
                "Wpb": Wpb,
                "Tout": t_out,
                "Tstd": t_std,
                "TH": th,
                "Ident": np.eye(128, dtype=np.float32),
            }
        )

    res = run_bass_kernel_spmd(
        nc, in_maps, core_ids=list(range(N_CORES)), trace=_want_trace
    )
    out = np.concatenate([res.results[c]["out_o"] for c in range(N_CORES)], axis=0)
    std = np.concatenate([res.results[c]["out_s"] for c in range(N_CORES)], axis=0)
    if _want_trace:
        kernel.last_results = res
    return out, std


# revision 13
# speedup vs baseline: 1.0027x; 1.0027x over previous
"""Trainium2 Bass kernel for nn_DGT_6485400616966 (soft decision tree forward).

Math (forward pass only):
  pred_z = x @ W_pred.T + b_pred                      [B, 1023]
  The straight-through/one-hot structure collapses: the output depends only on
  the argmax leaf of the tree AND layer, which equals a 10-level tree descent
  following sign(pred_z) at visited nodes (left if z >= 0).
  out = softmax(W_or[:, leaf]) ; std = clip(action_stds[:, leaf], -20, 2)

Device algorithm per core (8192 samples, data-parallel over 8 cores):
  1. PE: z = x @ W_pred.T in three fp32r passes (xh@wh + xh@wl + xl@wh) where
     hi/lo are an exact e8m11 split of the fp32 operands (fp32r on HW is
     e8m11; one pass alone flips ~38 argmax rows, three passes flip none).
     x tiles are the stationary operand; W^T columns are the moving operand,
     in two 512-wide halves (N=512 keeps full fp32r rate), each half a
     separate PSUM tile so eviction starts after 6 matmuls.
  2. Eviction PSUM->SBUF per (btile, half): u = (z < 0) as fp16, contiguous
     writes. Split ~5:1 ACT saturated-sigmoid (Sigmoid(-1e30*z) is exactly
     {0,1}) / DVE tensor_scalar is_lt.
  3. DVE: bottom-up tree collapse r = r_e + u*(K + r_o - r_e), K = 2^(i-9),
     in fp16 on [128, NB, 2^i] chunk tensors. Node columns are stored
     BIT-REVERSED within each level so r_e / r_o are contiguous halves:
     every op is a packed-SBUF-fp16 scalar_tensor_tensor -> 4x DVE mode.
     The collapsed r is leaf/512; the x512 is folded into the int16 cast.
  4. GPSIMD ap_gather per chunk: table lookup T[class, leaf] with the 16
     classes replicated on partitions (tables bit-reversal-permuted on host;
     host pre-permutes rows by pi(p)=8*(p%16)+p//16 so indices are already
     wrapped and outputs land in natural order).
  5. PE transpose of gathered [128, 128] blocks (one block per 8 btiles,
     emitted two chunks late so the in-order PE queue never stalls) +
     contiguous DMA out.
  Schedule: a few junk warmup matmuls run during the initial DMA so the PE
  HAM clock gate is released before the real stream starts; weight DMAs are
  issued split per 512-column half in first-use order; chunk sizes are
  [2,6,8,8,8,8,8,8,6,2] so the pipeline starts early and drains fast.
"""

import sys

for _p in ("/opt/trn_rl_repo",):
    if _p not in sys.path:
        sys.path.insert(0, _p)

from contextlib import ExitStack

import numpy as np

import concourse.bacc as bacc
import concourse.bass as bass
import concourse.tile as tile
from concourse import mybir
from concourse.bass_utils import run_bass_kernel_spmd

HEIGHT = 10
IN_DIM = 256
OUT_DIM = 16
BATCH = 65536
N_CORES = 8
B_LOC = BATCH // N_CORES          # 8192 samples per core
NT = B_LOC // 128                 # 64 batch tiles of 128 samples
NODES = 1024                      # 1023 real + 1 pad
F32 = mybir.dt.float32
F32R = mybir.dt.float32r
BF16 = mybir.dt.bfloat16
FP16 = mybir.dt.float16
I16 = mybir.dt.int16

CHUNKS = [2, 6, 12, 12, 12, 12, 6, 2]     # btiles per chunk; sums to NT
NBMAX = max(CHUNKS)
N_WARM = 16                               # junk matmuls to release HAM gate
# kb bias-constant layout: value 2^(i-10) at [KBOFF[i], KBOFF[i]+2^(i-1))
# biases the right half of the level-i result; [256,512) biases level 9.
KBOFF = {8: 0, 7: 128, 6: 192}
STT_LEVELS = 5  # levels < STT_LEVELS use the fused stt form (tiny arrays)


def _build(nc, use_sign_path: bool):
    """Emit the per-core program. use_sign_path=True assumes b_pred == 0."""
    xTh = nc.dram_tensor("xTh", [IN_DIM, B_LOC], F32R, kind="ExternalInput")
    xTl = nc.dram_tensor("xTl", [IN_DIM, B_LOC], BF16, kind="ExternalInput")
    Wph = nc.dram_tensor("Wph", [IN_DIM, NODES], F32R, kind="ExternalInput")
    Wpl = nc.dram_tensor("Wpl", [IN_DIM, NODES], F32R, kind="ExternalInput")
    Wpb = nc.dram_tensor("Wpb", [IN_DIM, NODES], BF16, kind="ExternalInput")
    Tout = nc.dram_tensor("Tout", [128, NODES], F32, kind="ExternalInput")
    Tstd = nc.dram_tensor("Tstd", [128, NODES], F32, kind="ExternalInput")
    TH = nc.dram_tensor("TH", [128, NODES], F32, kind="ExternalInput")
    Ident = nc.dram_tensor("Ident", [128, 128], F32, kind="ExternalInput")
    out_o = nc.dram_tensor("out_o", [B_LOC, OUT_DIM], F32, kind="ExternalOutput")
    out_s = nc.dram_tensor("out_s", [B_LOC, OUT_DIM], F32, kind="ExternalOutput")

    with tile.TileContext(nc) as tc, ExitStack() as ctx:
        consts = ctx.enter_context(tc.tile_pool(name="consts", bufs=1))
        xpool = ctx.enter_context(tc.tile_pool(name="xpool", bufs=2))
        spool = ctx.enter_context(tc.tile_pool(name="spool", bufs=2))
        rpool = ctx.enter_context(tc.tile_pool(name="rpool", bufs=3))
        dpool = ctx.enter_context(tc.tile_pool(name="dpool", bufs=3))
        zpool = ctx.enter_context(
            tc.tile_pool(name="zpool", bufs=6, space=bass.MemorySpace.PSUM)
        )
        tpool = ctx.enter_context(
            tc.tile_pool(name="tpool", bufs=2, space=bass.MemorySpace.PSUM)
        )

        # weight tiles split per (ktile kk, 512-col half h) for fine-grained
        # DMA deps: the first matmul only waits on one 256 KB quarter.
        wh = [[consts.tile([128, 512], F32R, tag=f"wh{k}{h}", name=f"wh{k}{h}")
               for h in range(2)] for k in range(2)]
        wl = [[consts.tile([128, 512], F32R, tag=f"wl{k}{h}", name=f"wl{k}{h}")
               for h in range(2)] for k in range(2)]
        wb = [[consts.tile([128, 512], BF16, tag=f"wb{k}{h}", name=f"wb{k}{h}")
               for h in range(2)] for k in range(2)]
        t_out = consts.tile([128, NODES], F32)
        t_std = consts.tile([128, NODES], F32)
        ident = consts.tile([128, 128], F32)
        wfake = consts.tile([128, 512], BF16)
        kbias = consts.tile([128, NBMAX, 512], FP16)
        th = None
        if not use_sign_path:
            th = consts.tile([128, NODES], F32)

        leaf_all = consts.tile([128, NT], FP16)
        leaf_i16 = consts.tile([128, NT], I16)
        r_out = consts.tile([128, NODES], F32)
        r_std = consts.tile([128, NODES], F32)

        o_view = out_o.rearrange("(t p f) c -> t p (f c)", t=8, p=128, f=8)
        s_view = out_s.rearrange("(t p f) c -> t p (f c)", t=8, p=128, f=8)

        A = mybir.AluOpType

        def dma_w(kk, h):
            ns = slice(512 * h, 512 * (h + 1))
            ks = slice(128 * kk, 128 * (kk + 1))
            nc.sync.dma_start(out=wh[kk][h], in_=Wph[ks, ns])
            nc.sync.dma_start(out=wl[kk][h], in_=Wpl[ks, ns])
            nc.sync.dma_start(out=wb[kk][h], in_=Wpb[ks, ns])

        # --- head: first weight quarter, then PE warmup on junk data ---
        nc.vector.memset(wfake, 0.0)
        for i, off in KBOFF.items():
            nc.vector.memset(kbias[:, :, off : off + (1 << (i - 1))],
                             float(2.0 ** (i - 10)))
        nc.vector.memset(kbias[:, :, 256:512], 0.5)
        nc.sync.dma_start(out=wh[0][0], in_=Wph[0:128, 0:512])
        zw = zpool.tile([128, 512], F32, tag="z")
        for i in range(N_WARM):
            nc.tensor.matmul(
                zw[:, 0:256], wfake[:, 0:128], wfake[:, 0:256],
                start=(i == 0), stop=(i == N_WARM - 1),
            )

        starts = np.cumsum([0] + CHUNKS).tolist()
        n_blocks = NT // 8
        blk_last_chunk = {}
        for b in range(n_blocks):
            for c, (t0, nb) in enumerate(zip(starts, CHUNKS)):
                if t0 < 8 * (b + 1) and t0 + nb > 8 * b:
                    blk_last_chunk[b] = c
        blocks_emitted = [False] * n_blocks

        def emit_block(b):
            # transpose block b's gathered [128, 128] table columns, DMA out
            blocks_emitted[b] = True
            rs_ = slice(128 * b, 128 * (b + 1))
            for rbuf, dview in ((r_out, o_view), (r_std, s_view)):
                pt = tpool.tile([128, 128], F32, tag="t", name="pt")
                nc.tensor.transpose(pt, rbuf[:, rs_], ident)
                rt = xpool.tile([128, 128], F32, tag="rt", name="rt", bufs=2)
                nc.vector.tensor_copy(out=rt, in_=pt)
                nc.sync.dma_start(out=dview[b], in_=rt)

        ev = 0  # eviction round-robin counter
        for c, (t0, NB) in enumerate(zip(starts, CHUNKS)):
            s_chunk = spool.tile([128, NBMAX, NODES], FP16, tag="s")
            # stage x for this chunk: [128, 128*NB] per ktile/half
            hs = slice(128 * t0, 128 * (t0 + NB))
            xh = [xpool.tile([128, 128 * NBMAX], F32R, tag=f"xh{kk}",
                             name=f"xh{kk}", bufs=2) for kk in range(2)]
            xl = [xpool.tile([128, 128 * NBMAX], BF16, tag=f"xl{kk}",
                             name=f"xl{kk}", bufs=2) for kk in range(2)]
            for kk in range(2):
                ks = slice(128 * kk, 128 * (kk + 1))
                nc.sync.dma_start(out=xh[kk][:, : 128 * NB], in_=xTh[ks, hs])
                nc.sync.dma_start(out=xl[kk][:, : 128 * NB], in_=xTl[ks, hs])
            if c == 0:
                # remaining weights in first-use order, tables last
                nc.sync.dma_start(out=wl[0][0], in_=Wpl[0:128, 0:512])
                nc.sync.dma_start(out=wb[0][0], in_=Wpb[0:128, 0:512])
                dma_w(1, 0)
                nc.sync.dma_start(out=wh[0][1], in_=Wph[0:128, 512:1024])
                nc.sync.dma_start(out=wl[0][1], in_=Wpl[0:128, 512:1024])
                nc.sync.dma_start(out=wb[0][1], in_=Wpb[0:128, 512:1024])
                dma_w(1, 1)
                nc.sync.dma_start(out=t_out, in_=Tout[:, :])
                nc.sync.dma_start(out=t_std, in_=Tstd[:, :])
                nc.sync.dma_start(out=ident, in_=Ident[:, :])
                if th is not None:
                    nc.sync.dma_start(out=th, in_=TH[:, :])

            for k in range(NB):
                kb = slice(128 * k, 128 * (k + 1))
                for h in range(2):
                    ns = slice(512 * h, 512 * (h + 1))
                    z = zpool.tile([128, 512], F32, tag="z")
                    pair = 0
                    for kk in range(2):
                        for lhs, rhs in (
                            (xh[kk], wh[kk][h]),
                            (xh[kk], wl[kk][h]),
                            (xl[kk], wb[kk][h]),
                        ):
                            nc.tensor.matmul(
                                z, lhs[:, kb], rhs,
                                start=(pair == 0), stop=(pair == 5),
                            )
                            pair += 1
                    # u = (z < -b_pred); contiguous fp16 write of one half
                    if use_sign_path:
                        if ev % 6 != 5:
                            nc.scalar.activation(
                                out=s_chunk[:, k, ns], in_=z,
                                func=mybir.ActivationFunctionType.Sigmoid,
                                scale=-1e30,
                            )
                        else:
                            nc.vector.tensor_scalar(
                                out=s_chunk[:, k, ns], in0=z,
                                scalar1=0.0, scalar2=None, op0=A.is_lt,
                            )
                        ev += 1
                    else:
                        nc.vector.tensor_tensor(
                            out=s_chunk[:, k, ns], in0=z, in1=th[:, ns],
                            op=A.is_lt,
                        )

            # ---- bottom-up collapse; bit-reversed node layout makes r_e /
            # r_o contiguous halves. Big levels use single-ALU tensor_tensor
            # ops (fastest DVE path) with the +K bias pre-applied to the
            # right half via the kb constant; tiny levels use the fused stt.
            nc.vector.tensor_tensor(
                out=s_chunk[:, :NB, 767:1023], in0=s_chunk[:, :NB, 767:1023],
                in1=kbias[:, :NB, 256:512], op=A.add,
            )
            r_prev = s_chunk[:, :NB, 511:1023]
            for i in range(8, -1, -1):
                n = 1 << i
                u_i = s_chunk[:, :NB, n - 1 : 2 * n - 1]
                r_e = r_prev[:, :, :n]
                r_o = r_prev[:, :, n : 2 * n]
                d_t = dpool.tile([128, NB, n], FP16, tag="d")
                if i >= STT_LEVELS:
                    nc.vector.tensor_tensor(
                        out=d_t, in0=r_o, in1=r_e, op=A.subtract,
                    )
                else:
                    nc.vector.scalar_tensor_tensor(
                        out=d_t, in0=r_o, scalar=float(2.0 ** (i - 9)),
                        in1=r_e, op0=A.add, op1=A.subtract,
                    )
                nc.vector.tensor_tensor(
                    out=d_t, in0=d_t, in1=u_i, op=A.mult,
                )
                if i > 0:
                    r_t = rpool.tile([128, NB, n], FP16, tag="r")
                    nc.vector.tensor_tensor(
                        out=r_t, in0=d_t, in1=r_e, op=A.add,
                    )
                    if i in KBOFF:
                        off = KBOFF[i]
                        nc.vector.tensor_tensor(
                            out=r_t[:, :, n // 2 :], in0=r_t[:, :, n // 2 :],
                            in1=kbias[:, :NB, off : off + n // 2], op=A.add,
                        )
                    r_prev = r_t
                else:
                    nc.vector.tensor_tensor(
                        out=leaf_all[:, t0 : t0 + NB],
                        in0=d_t[:, :, 0], in1=r_e[:, :, 0], op=A.add,
                    )

            # leaf/512 -> int16 leaf index (x512 folded into the cast)
            cslice = slice(t0, t0 + NB)
            nc.vector.tensor_scalar(
                out=leaf_i16[:, cslice], in0=leaf_all[:, cslice],
                scalar1=512.0, scalar2=None, op0=A.mult,
            )
            # table gathers: R[16g+cls, j] = T[cls, leaf(sample 8j+g)]
            rs = slice(16 * t0, 16 * (t0 + NB))
            for tbl, rbuf in ((t_out, r_out), (t_std, r_std)):
                nc.gpsimd.ap_gather(
                    out_ap=rbuf[:, rs], in_ap=tbl,
                    idxs_ap=leaf_i16[:, cslice],
                    channels=128, num_elems=NODES, d=1, num_idxs=16 * NB,
                )
            for b in range(n_blocks):
                if not blocks_emitted[b] and blk_last_chunk[b] + 2 <= c:
                    emit_block(b)

        for b in range(n_blocks):
            if not blocks_emitted[b]:
                emit_block(b)

    nc.compile()
    return nc


_CACHE = {}


def _get_nc(use_sign_path: bool):
    key = use_sign_path
    if key not in _CACHE:
        nc = bacc.Bacc("TRN2", target_bir_lowering=False, debug=False)
        _CACHE[key] = _build(nc, use_sign_path)
    return _CACHE[key]


# Within each 128-row block, device partition p holds sample row PERM[p].
# PERM aligns the collapse output with ap_gather's wrapped index layout and
# makes the final outputs land in natural row order.
PERM = np.array([8 * (p % 16) + p // 16 for p in range(128)], dtype=np.int64)


def _bitrev(n_bits):
    """Bit-reversal permutation of range(2**n_bits)."""
    n = 1 << n_bits
    out = np.zeros(n, dtype=np.int64)
    for j in range(n):
        r = 0
        for b in range(n_bits):
            r |= ((j >> b) & 1) << (n_bits - 1 - b)
        out[j] = r
    return out


def _node_colperm():
    """colperm[stored] = natural padded-column; stored order is bit-reversed
    within each level so the device collapse reads contiguous halves."""
    perm = np.arange(NODES, dtype=np.int64)
    for lev in range(HEIGHT):
        off = (1 << lev) - 1
        perm[off : off + (1 << lev)] = off + _bitrev(lev)
    return perm


COLPERM = _node_colperm()
LEAFPERM = _bitrev(HEIGHT)  # natural leaf = LEAFPERM[stored leaf]


def _e8m11(x):
    """Round fp32 to the HW fp32r format (8-bit exp, 11-bit mantissa, RNE)."""
    u = np.ascontiguousarray(x, np.float32).view(np.uint32)
    low = u & np.uint32(0xFFF)
    base = u & np.uint32(0xFFFFF000)
    add = (low > 0x800) | ((low == 0x800) & ((u >> 12) & 1).astype(bool))
    return (base + np.where(add, np.uint32(0x1000), np.uint32(0))).view(np.float32)


def _split_hi_lo(a, lo_bf16=False):
    hi = _e8m11(a)
    lo = (a - hi).astype(np.float32)  # exactly e8m11-representable
    if lo_bf16:
        import ml_dtypes
        lo = lo.astype(ml_dtypes.bfloat16)
    return hi, lo


def _shard_xT(x_shard):
    """[8192, 256] sample rows -> permuted, transposed [256, 8192] device input."""
    xp = x_shard.reshape(NT, 128, IN_DIM)[:, PERM, :].reshape(B_LOC, IN_DIM)
    return np.ascontiguousarray(xp.T)


def _prepare(x, W_pred, b_pred, W_or, action_stds):
    x = np.ascontiguousarray(x, dtype=np.float32)
    W_pred = np.asarray(W_pred, dtype=np.float32)
    b_pred = np.asarray(b_pred, dtype=np.float32)
    W_or = np.asarray(W_or, dtype=np.float32)
    action_stds = np.asarray(action_stds, dtype=np.float32)

    n_int = 2**HEIGHT - 1
    Wp = np.zeros((IN_DIM, NODES), np.float32)
    Wp[:, :n_int] = W_pred.T
    Wp = Wp[:, COLPERM]  # bit-reversed node storage order per level
    Wph, Wpl = _split_hi_lo(Wp)
    import ml_dtypes
    Wpb = Wph.astype(ml_dtypes.bfloat16)
    # softmax over classes per leaf column, tables in stored-leaf order
    m = W_or.max(axis=0, keepdims=True)
    e = np.exp(W_or - m)
    t_out16 = (e / e.sum(axis=0, keepdims=True)).astype(np.float32)[:, LEAFPERM]
    t_std16 = np.clip(action_stds, -20.0, 2.0).astype(np.float32)[:, LEAFPERM]
    t_out = np.tile(t_out16, (8, 1))  # [128, 1024]
    t_std = np.tile(t_std16, (8, 1))
    th16 = np.zeros((NODES,), np.float32)
    th16[:n_int] = -b_pred
    th16 = th16[COLPERM]
    th = np.tile(th16[None, :], (128, 1))
    return x, Wph, Wpl, Wpb, t_out, t_std, th, bool(np.any(b_pred != 0.0))


def kernel(x, W_pred, b_pred, W_or, action_stds, _want_trace=False):
    x, Wph, Wpl, Wpb, t_out, t_std, th, b_nonzero = _prepare(
        x, W_pred, b_pred, W_or, action_stds
    )
    nc = _get_nc(use_sign_path=not b_nonzero)

    in_maps = []
    for c in range(N_CORES):
        shard = x[c * B_LOC : (c + 1) * B_LOC]
        xt = _shard_xT(shard)
        xth, xtl = _split_hi_lo(xt, lo_bf16=True)
        in_maps.append(
            {
                "xTh": xth,
                "xTl": xtl,
                "Wph": Wph,
                "Wpl": Wpl,
                "Wpb": Wpb,
                "Tout": t_out,
                "Tstd": t_std,
                "TH": th,
                "Ident": np.eye(128, dtype=np.float32),
            }
        )

    res = run_bass_kernel_spmd(
        nc, in_maps, core_ids=list(range(N_CORES)), trace=_want_trace
    )
    out = np.concatenate([res.results[c]["out_o"] for c in range(N_CORES)], axis=0)
    std = np.concatenate([res.results[c]["out_s"] for c in range(N_CORES)], axis=0)
    if _want_trace:
        kernel.last_results = res
    return out, std


# revision 15
# speedup vs baseline: 1.0032x; 1.0005x over previous
"""Trainium2 Bass kernel for nn_DGT_6485400616966 (soft decision tree forward).

Math (forward pass only):
  pred_z = x @ W_pred.T + b_pred                      [B, 1023]
  The straight-through/one-hot structure collapses: the output depends only on
  the argmax leaf of the tree AND layer, which equals a 10-level tree descent
  following sign(pred_z) at visited nodes (left if z >= 0).
  out = softmax(W_or[:, leaf]) ; std = clip(action_stds[:, leaf], -20, 2)

Device algorithm per core (8192 samples, data-parallel over 8 cores):
  1. PE: z = x @ W_pred.T in three fp32r passes (xh@wh + xh@wl + xl@wh) where
     hi/lo are an exact e8m11 split of the fp32 operands (fp32r on HW is
     e8m11; one pass alone flips ~38 argmax rows, three passes flip none).
     x tiles are the stationary operand; W^T columns are the moving operand,
     in two 512-wide halves (N=512 keeps full fp32r rate), each half a
     separate PSUM tile so eviction starts after 6 matmuls.
  2. Eviction PSUM->SBUF per (btile, half): u = (z < 0) as fp16, contiguous
     writes. Split ~5:1 ACT saturated-sigmoid (Sigmoid(-1e30*z) is exactly
     {0,1}) / DVE tensor_scalar is_lt.
  3. DVE: bottom-up tree collapse r = r_e + u*(K + r_o - r_e), K = 2^(i-9),
     in fp16 on [128, NB, 2^i] chunk tensors. Node columns are stored
     BIT-REVERSED within each level so r_e / r_o are contiguous halves:
     every op is a packed-SBUF-fp16 scalar_tensor_tensor -> 4x DVE mode.
     The collapsed r is leaf/512; the x512 is folded into the int16 cast.
  4. GPSIMD ap_gather per chunk: table lookup T[class, leaf] with the 16
     classes replicated on partitions (tables bit-reversal-permuted on host;
     host pre-permutes rows by pi(p)=8*(p%16)+p//16 so indices are already
     wrapped and outputs land in natural order).
  5. PE transpose of gathered [128, 128] blocks (one block per 8 btiles,
     emitted two chunks late so the in-order PE queue never stalls) +
     contiguous DMA out.
  Schedule: a few junk warmup matmuls run during the initial DMA so the PE
  HAM clock gate is released before the real stream starts; weight DMAs are
  issued split per 512-column half in first-use order; chunk sizes are
  [2,6,8,8,8,8,8,8,6,2] so the pipeline starts early and drains fast.
"""

import sys

for _p in ("/opt/trn_rl_repo",):
    if _p not in sys.path:
        sys.path.insert(0, _p)

from contextlib import ExitStack

import numpy as np

import concourse.bacc as bacc
import concourse.bass as bass
import concourse.tile as tile
from concourse import mybir
from concourse.bass_utils import run_bass_kernel_spmd

HEIGHT = 10
IN_DIM = 256
OUT_DIM = 16
BATCH = 65536
N_CORES = 8
B_LOC = BATCH // N_CORES          # 8192 samples per core
NT = B_LOC // 128                 # 64 batch tiles of 128 samples
NODES = 1024                      # 1023 real + 1 pad
F32 = mybir.dt.float32
F32R = mybir.dt.float32r
BF16 = mybir.dt.bfloat16
FP16 = mybir.dt.float16
I16 = mybir.dt.int16

CHUNKS = [2, 6, 12, 12, 12, 14, 4, 2]     # btiles per chunk; sums to NT
NBMAX = max(CHUNKS)
N_WARM = 16                               # junk matmuls to release HAM gate
# kb bias-constant layout: value 2^(i-10) at [KBOFF[i], KBOFF[i]+2^(i-1))
# biases the right half of the level-i result; [256,512) biases level 9.
KBOFF = {8: 0, 7: 128, 6: 192}
STT_LEVELS = 5  # levels < STT_LEVELS use the fused stt form (tiny arrays)


def _build(nc, use_sign_path: bool):
    """Emit the per-core program. use_sign_path=True assumes b_pred == 0."""
    xTh = nc.dram_tensor("xTh", [IN_DIM, B_LOC], F32R, kind="ExternalInput")
    xTl = nc.dram_tensor("xTl", [IN_DIM, B_LOC], BF16, kind="ExternalInput")
    Wph = nc.dram_tensor("Wph", [IN_DIM, NODES], F32R, kind="ExternalInput")
    Wpl = nc.dram_tensor("Wpl", [IN_DIM, NODES], F32R, kind="ExternalInput")
    Wpb = nc.dram_tensor("Wpb", [IN_DIM, NODES], BF16, kind="ExternalInput")
    Tout = nc.dram_tensor("Tout", [128, NODES], F32, kind="ExternalInput")
    Tstd = nc.dram_tensor("Tstd", [128, NODES], F32, kind="ExternalInput")
    TH = nc.dram_tensor("TH", [128, NODES], F32, kind="ExternalInput")
    Ident = nc.dram_tensor("Ident", [128, 128], F32, kind="ExternalInput")
    out_o = nc.dram_tensor("out_o", [B_LOC, OUT_DIM], F32, kind="ExternalOutput")
    out_s = nc.dram_tensor("out_s", [B_LOC, OUT_DIM], F32, kind="ExternalOutput")

    with tile.TileContext(nc) as tc, ExitStack() as ctx:
        consts = ctx.enter_context(tc.tile_pool(name="consts", bufs=1))
        xpool = ctx.enter_context(tc.tile_pool(name="xpool", bufs=2))
        spool = ctx.enter_context(tc.tile_pool(name="spool", bufs=2))
        rpool = ctx.enter_context(tc.tile_pool(name="rpool", bufs=3))
        dpool = ctx.enter_context(tc.tile_pool(name="dpool", bufs=3))
        zpool = ctx.enter_context(
            tc.tile_pool(name="zpool", bufs=5, space=bass.MemorySpace.PSUM)
        )
        tpool = ctx.enter_context(
            tc.tile_pool(name="tpool", bufs=3, space=bass.MemorySpace.PSUM)
        )

        # weight tiles split per (ktile kk, 512-col half h) for fine-grained
        # DMA deps: the first matmul only waits on one 256 KB quarter.
        wh = [[consts.tile([128, 512], F32R, tag=f"wh{k}{h}", name=f"wh{k}{h}")
               for h in range(2)] for k in range(2)]
        wl = [[consts.tile([128, 512], F32R, tag=f"wl{k}{h}", name=f"wl{k}{h}")
               for h in range(2)] for k in range(2)]
        wb = [[consts.tile([128, 512], BF16, tag=f"wb{k}{h}", name=f"wb{k}{h}")
               for h in range(2)] for k in range(2)]
        t_out = consts.tile([128, NODES], F32)
        t_std = consts.tile([128, NODES], F32)
        ident = consts.tile([128, 128], F32)
        wfake = consts.tile([128, 512], BF16)
        kbias = consts.tile([128, NBMAX, 512], FP16)
        th = None
        if not use_sign_path:
            th = consts.tile([128, NODES], F32)

        leaf_all = consts.tile([128, NT], FP16)
        leaf_i16 = consts.tile([128, NT], I16)
        r_out = consts.tile([128, NODES], F32)
        r_std = consts.tile([128, NODES], F32)

        o_view = out_o.rearrange("(t p f) c -> t p (f c)", t=8, p=128, f=8)
        s_view = out_s.rearrange("(t p f) c -> t p (f c)", t=8, p=128, f=8)

        A = mybir.AluOpType

        def dma_w(kk, h):
            ns = slice(512 * h, 512 * (h + 1))
            ks = slice(128 * kk, 128 * (kk + 1))
            nc.sync.dma_start(out=wh[kk][h], in_=Wph[ks, ns])
            nc.sync.dma_start(out=wl[kk][h], in_=Wpl[ks, ns])
            nc.sync.dma_start(out=wb[kk][h], in_=Wpb[ks, ns])

        # --- head: first weight quarter, then PE warmup on junk data ---
        nc.vector.memset(wfake, 0.0)
        for i, off in KBOFF.items():
            nc.vector.memset(kbias[:, :, off : off + (1 << (i - 1))],
                             float(2.0 ** (i - 10)))
        nc.vector.memset(kbias[:, :, 256:512], 0.5)
        nc.sync.dma_start(out=wh[0][0], in_=Wph[0:128, 0:512])
        zw = zpool.tile([128, 512], F32, tag="z")
        for i in range(N_WARM):
            nc.tensor.matmul(
                zw[:, 0:256], wfake[:, 0:128], wfake[:, 0:256],
                start=(i == 0), stop=(i == N_WARM - 1),
            )

        starts = np.cumsum([0] + CHUNKS).tolist()
        n_blocks = NT // 8
        blk_last_chunk = {}
        for b in range(n_blocks):
            for c, (t0, nb) in enumerate(zip(starts, CHUNKS)):
                if t0 < 8 * (b + 1) and t0 + nb > 8 * b:
                    blk_last_chunk[b] = c
        blocks_emitted = [False] * n_blocks

        def emit_block(b):
            # transpose block b's gathered [128, 128] table columns, DMA out
            blocks_emitted[b] = True
            rs_ = slice(128 * b, 128 * (b + 1))
            for rbuf, dview in ((r_out, o_view), (r_std, s_view)):
                pt = tpool.tile([128, 128], F32, tag="t", name="pt")
                nc.tensor.transpose(pt, rbuf[:, rs_], ident)
                rt = xpool.tile([128, 128], F32, tag="rt", name="rt", bufs=2)
                nc.scalar.copy(out=rt, in_=pt)
                nc.sync.dma_start(out=dview[b], in_=rt)

        ev = 0  # eviction round-robin counter
        for c, (t0, NB) in enumerate(zip(starts, CHUNKS)):
            s_chunk = spool.tile([128, NBMAX, NODES], FP16, tag="s")
            # stage x for this chunk: [128, 128*NB] per ktile/half
            hs = slice(128 * t0, 128 * (t0 + NB))
            xh = [xpool.tile([128, 128 * NBMAX], F32R, tag=f"xh{kk}",
                             name=f"xh{kk}", bufs=2) for kk in range(2)]
            xl = [xpool.tile([128, 128 * NBMAX], BF16, tag=f"xl{kk}",
                             name=f"xl{kk}", bufs=2) for kk in range(2)]
            for kk in range(2):
                ks = slice(128 * kk, 128 * (kk + 1))
                nc.sync.dma_start(out=xh[kk][:, : 128 * NB], in_=xTh[ks, hs])
                nc.sync.dma_start(out=xl[kk][:, : 128 * NB], in_=xTl[ks, hs])
            if c == 0:
                # remaining weights in first-use order, tables last
                nc.sync.dma_start(out=wl[0][0], in_=Wpl[0:128, 0:512])
                nc.sync.dma_start(out=wb[0][0], in_=Wpb[0:128, 0:512])
                dma_w(1, 0)
                nc.sync.dma_start(out=wh[0][1], in_=Wph[0:128, 512:1024])
                nc.sync.dma_start(out=wl[0][1], in_=Wpl[0:128, 512:1024])
                nc.sync.dma_start(out=wb[0][1], in_=Wpb[0:128, 512:1024])
                dma_w(1, 1)
                nc.sync.dma_start(out=t_out, in_=Tout[:, :])
                nc.sync.dma_start(out=t_std, in_=Tstd[:, :])
                nc.sync.dma_start(out=ident, in_=Ident[:, :])
                if th is not None:
                    nc.sync.dma_start(out=th, in_=TH[:, :])

            for k in range(NB):
                kb = slice(128 * k, 128 * (k + 1))
                for h in range(2):
                    ns = slice(512 * h, 512 * (h + 1))
                    z = zpool.tile([128, 512], F32, tag="z")
                    pair = 0
                    for kk in range(2):
                        for lhs, rhs in (
                            (xh[kk], wh[kk][h]),
                            (xh[kk], wl[kk][h]),
                            (xl[kk], wb[kk][h]),
                        ):
                            nc.tensor.matmul(
                                z, lhs[:, kb], rhs,
                                start=(pair == 0), stop=(pair == 5),
                            )
                            pair += 1
                    # u = (z < -b_pred); contiguous fp16 write of one half
                    if use_sign_path:
                        if ev % 6 != 5:
                            nc.scalar.activation(
                                out=s_chunk[:, k, ns], in_=z,
                                func=mybir.ActivationFunctionType.Sigmoid,
                                scale=-1e30,
                            )
                        else:
                            nc.vector.tensor_scalar(
                                out=s_chunk[:, k, ns], in0=z,
                                scalar1=0.0, scalar2=None, op0=A.is_lt,
                            )
                        ev += 1
                    else:
                        nc.vector.tensor_tensor(
                            out=s_chunk[:, k, ns], in0=z, in1=th[:, ns],
                            op=A.is_lt,
                        )

            # ---- bottom-up collapse; bit-reversed node layout makes r_e /
            # r_o contiguous halves. Big levels use single-ALU tensor_tensor
            # ops (fastest DVE path) with the +K bias pre-applied to the
            # right half via the kb constant; tiny levels use the fused stt.
            nc.vector.tensor_tensor(
                out=s_chunk[:, :NB, 767:1023], in0=s_chunk[:, :NB, 767:1023],
                in1=kbias[:, :NB, 256:512], op=A.add,
            )
            r_prev = s_chunk[:, :NB, 511:1023]
            for i in range(8, -1, -1):
                n = 1 << i
                u_i = s_chunk[:, :NB, n - 1 : 2 * n - 1]
                r_e = r_prev[:, :, :n]
                r_o = r_prev[:, :, n : 2 * n]
                d_t = dpool.tile([128, NB, n], FP16, tag="d")
                if i >= STT_LEVELS:
                    nc.vector.tensor_tensor(
                        out=d_t, in0=r_o, in1=r_e, op=A.subtract,
                    )
                else:
                    nc.vector.scalar_tensor_tensor(
                        out=d_t, in0=r_o, scalar=float(2.0 ** (i - 9)),
                        in1=r_e, op0=A.add, op1=A.subtract,
                    )
                nc.vector.tensor_tensor(
                    out=d_t, in0=d_t, in1=u_i, op=A.mult,
                )
                if i > 0:
                    r_t = rpool.tile([128, NB, n], FP16, tag="r")
                    nc.vector.tensor_tensor(
                        out=r_t, in0=d_t, in1=r_e, op=A.add,
                    )
                    if i in KBOFF:
                        off = KBOFF[i]
                        nc.vector.tensor_tensor(
                            out=r_t[:, :, n // 2 :], in0=r_t[:, :, n // 2 :],
                            in1=kbias[:, :NB, off : off + n // 2], op=A.add,
                        )
                    r_prev = r_t
                else:
                    nc.vector.tensor_tensor(
                        out=leaf_all[:, t0 : t0 + NB],
                        in0=d_t[:, :, 0], in1=r_e[:, :, 0], op=A.add,
                    )

            # leaf/512 -> int16 leaf index (x512 folded into the cast)
            cslice = slice(t0, t0 + NB)
            nc.vector.tensor_scalar(
                out=leaf_i16[:, cslice], in0=leaf_all[:, cslice],
                scalar1=512.0, scalar2=None, op0=A.mult,
            )
            # table gathers: R[16g+cls, j] = T[cls, leaf(sample 8j+g)]
            rs = slice(16 * t0, 16 * (t0 + NB))
            for tbl, rbuf in ((t_out, r_out), (t_std, r_std)):
                nc.gpsimd.ap_gather(
                    out_ap=rbuf[:, rs], in_ap=tbl,
                    idxs_ap=leaf_i16[:, cslice],
                    channels=128, num_elems=NODES, d=1, num_idxs=16 * NB,
                )
            for b in range(n_blocks):
                if not blocks_emitted[b] and blk_last_chunk[b] + 1 <= c:
                    emit_block(b)

        for b in range(n_blocks):
            if not blocks_emitted[b]:
                emit_block(b)

    nc.compile()
    return nc


_CACHE = {}


def _get_nc(use_sign_path: bool):
    key = use_sign_path
    if key not in _CACHE:
        nc = bacc.Bacc("TRN2", target_bir_lowering=False, debug=False)
        _CACHE[key] = _build(nc, use_sign_path)
    return _CACHE[key]


# Within each 128-row block, device partition p holds sample row PERM[p].
# PERM aligns the collapse output with ap_gather's wrapped index layout and
# makes the final outputs land in natural row order.
PERM = np.array([8 * (p % 16) + p // 16 for p in range(128)], dtype=np.int64)


def _bitrev(n_bits):
    """Bit-reversal permutation of range(2**n_bits)."""
    n = 1 << n_bits
    out = np.zeros(n, dtype=np.int64)
    for j in range(n):
        r = 0
        for b in range(n_bits):
            r |= ((j >> b) & 1) << (n_bits - 1 - b)
        out[j] = r
    return out


def _node_colperm():
    """colperm[stored] = natural padded-column; stored order is bit-reversed
    within each level so the device collapse reads contiguous halves."""
    perm = np.arange(NODES, dtype=np.int64)
    for lev in range(HEIGHT):
        off = (1 << lev) - 1
        perm[off : off + (1 << lev)] = off + _bitrev(lev)
    return perm


COLPERM = _node_colperm()
LEAFPERM = _bitrev(HEIGHT)  # natural leaf = LEAFPERM[stored leaf]


def _e8m11(x):
    """Round fp32 to the HW fp32r format (8-bit exp, 11-bit mantissa, RNE)."""
    u = np.ascontiguousarray(x, np.float32).view(np.uint32)
    low = u & np.uint32(0xFFF)
    base = u & np.uint32(0xFFFFF000)
    add = (low > 0x800) | ((low == 0x800) & ((u >> 12) & 1).astype(bool))
    return (base + np.where(add, np.uint32(0x1000), np.uint32(0))).view(np.float32)


def _split_hi_lo(a, lo_bf16=False):
    hi = _e8m11(a)
    lo = (a - hi).astype(np.float32)  # exactly e8m11-representable
    if lo_bf16:
        import ml_dtypes
        lo = lo.astype(ml_dtypes.bfloat16)
    return hi, lo


def _shard_xT(x_shard):
    """[8192, 256] sample rows -> permuted, transposed [256, 8192] device input."""
    xp = x_shard.reshape(NT, 128, IN_DIM)[:, PERM, :].reshape(B_LOC, IN_DIM)
    return np.ascontiguousarray(xp.T)


def _prepare(x, W_pred, b_pred, W_or, action_stds):
    x = np.ascontiguousarray(x, dtype=np.float32)
    W_pred = np.asarray(W_pred, dtype=np.float32)
    b_pred = np.asarray(b_pred, dtype=np.float32)
    W_or = np.asarray(W_or, dtype=np.float32)
    action_stds = np.asarray(action_stds, dtype=np.float32)

    n_int = 2**HEIGHT - 1
    Wp = np.zeros((IN_DIM, NODES), np.float32)
    Wp[:, :n_int] = W_pred.T
    Wp = Wp[:, COLPERM]  # bit-reversed node storage order per level
    Wph, Wpl = _split_hi_lo(Wp)
    import ml_dtypes
    Wpb = Wph.astype(ml_dtypes.bfloat16)
    # softmax over classes per leaf column, tables in stored-leaf order
    m = W_or.max(axis=0, keepdims=True)
    e = np.exp(W_or - m)
    t_out16 = (e / e.sum(axis=0, keepdims=True)).astype(np.float32)[:, LEAFPERM]
    t_std16 = np.clip(action_stds, -20.0, 2.0).astype(np.float32)[:, LEAFPERM]
    t_out = np.tile(t_out16, (8, 1))  # [128, 1024]
    t_std = np.tile(t_std16, (8, 1))
    th16 = np.zeros((NODES,), np.float32)
    th16[:n_int] = -b_pred
    th16 = th16[COLPERM]
    th = np.tile(th16[None, :], (128, 1))
    return x, Wph, Wpl, Wpb, t_out, t_std, th, bool(np.any(b_pred != 0.0))


def kernel(x, W_pred, b_pred, W_or, action_stds, _want_trace=False):
    x, Wph, Wpl, Wpb, t_out, t_std, th, b_nonzero = _prepare(
        x, W_pred, b_pred, W_or, action_stds
    )
    nc = _get_nc(use_sign_path=not b_nonzero)

    in_maps = []
    for c in range(N_CORES):
        shard = x[c * B_LOC : (c + 1) * B_LOC]
        xt = _shard_xT(shard)
        xth, xtl = _split_hi_lo(xt, lo_bf16=True)
        in_maps.append(
            {
                "xTh": xth,
                "xTl": xtl,
                "Wph": Wph,
                "Wpl": Wpl,
                "Wpb": Wpb,
                "Tout": t_out,
                "Tstd": t_std,
                "TH": th,
                "Ident": np.eye(128, dtype=np.float32),
            }
        )

    res = run_bass_kernel_spmd(
        nc, in_maps, core_ids=list(range(N_CORES)), trace=_want_trace
    )
    out = np.concatenate([res.results[c]["out_o"] for c in range(N_CORES)], axis=0)
    std = np.concatenate([res.results[c]["out_s"] for c in range(N_CORES)], axis=0)
    if _want_trace:
        kernel.last_results = res
    return out, std
